# revision 1
# baseline (speedup 1.0000x reference)
"""Trainium2 Bass kernel for nn_BatchContrastLoss (InfoNCE-style contrastive loss).

Reference computation:
    sim[i,j]  = cos(que_i, ans_j)            (eps-guarded norms)
    logits    = sim / 0.07
    loss      = -mean_i(log_softmax(logits, axis=1)[i,i])

Sharding: data-parallel over rows of que across 8 NeuronCores. Each core
computes its [512, 4096] logits slab against the full ans batch, does local
row-wise sum-exp (no max subtraction needed: |logits| <= 1/0.07 so exp stays
comfortably inside fp32 range), and emits per-row softmax denominators plus
diagonal logits. The host takes log + mean (the "all-reduce" of the hint).

Per-core design notes:
  - que^T slab and ans^T arrive d-major so the D=1024 contraction sits on
    the partition axis, pre-paired [128, 2, *] for DoubleRow fp8e4m3
    matmuls (2 weights/cell, K=256 per instruction). PSUM accumulation is
    fp32; quantization error largely cancels in the 4096-term mean (HW
    measured ~3e-6 relative on the loss).
  - Every core computes ALL 4096 ans norms itself (square + ones-matmul
    partition-reduction per streamed chunk). This is redundant across cores
    but strictly local: a cross-core AllGather measured ~50-70us of
    rank-skew stall here, far worse than the ~17us of redundant compute.
  - 1/norm uses exp(-0.5*ln(x)) on ScalarE (both functions live in one
    activation table set; DVE reciprocal is iterative and ~5x slower).
  - psum drain: DVE multiply by the broadcast column scale, then ScalarE
    Exp with per-partition row scale and fused row-sum accumulation.
"""

import numpy as np

import concourse.bass as bass
import concourse.mybir as mybir
import concourse.tile as tile
from concourse import bacc
from concourse.bass_utils import run_bass_kernel_spmd

# Problem constants (self-contained; the harness provides only the inputs).
B = 4096  # rows of que_batch / ans_batch
D = 1024  # feature dim
NCORES = 8
NB = B // NCORES  # local que rows per core = 512
P = 128  # SBUF partitions
KT = D // P  # 8 contraction k-tiles
NW = 512  # column chunk width (one fp32 PSUM bank)
NCH = B // NW  # 8 column chunks
MT = NB // P  # 4 row tiles of 128
GAMA = 0.07
EPS = 1e-8

F32 = mybir.dt.float32
F32R = mybir.dt.float32r  # fp32 truncated to FP22 in the PE (single pass)
BF16 = mybir.dt.bfloat16
FP8 = mybir.dt.float8e4  # e4m3: matmul operands; DoubleRow packs 2 weights/cell
DR = mybir.MatmulPerfMode.DoubleRow
KT2 = KT // 2  # k-pair tiles for DoubleRow (each matmul contracts 256 dims)
AF = mybir.ActivationFunctionType



def _patch_act_tables():
    """Force all Square/Ln/Exp activations into the one table set that
    contains all three (natural_log_exp_and_others). The stock picker
    chooses the first set containing each function, which alternates
    between exp_and_others and natural_log and cost ~21 table reloads
    (~27us) per kernel. Stripping those funcs from every other set (the
    list is only used for set selection; ids still index act_info.json)
    collapses this to a single load."""
    import concourse.bacc as bacc_mod
    from concourse.hw_specs import get_activation_tables as orig

    if getattr(bacc_mod, "_act_tables_patched", False):
        return

    def patched(arch):
        tabs = orig(arch)
        target = "natural_log_exp_and_others"
        if target in tabs:
            strip = {
                mybir.ActivationFunctionType.Exp,
                mybir.ActivationFunctionType.Ln,
                mybir.ActivationFunctionType.Square,
            }
            for name, fns in tabs.items():
                if name != target:
                    tabs[name] = fns - strip
        return tabs

    bacc_mod.get_activation_tables = patched
    bacc_mod._act_tables_patched = True


def _build_program():
    _patch_act_tables()
    nc = bacc.Bacc(
        "TRN2", target_bir_lowering=False, debug=False, num_devices=NCORES
    )

    qT = nc.dram_tensor("qT", [D, NB], FP8, kind="ExternalInput").ap()
    aT = nc.dram_tensor("aT", [D, B], FP8, kind="ExternalInput").ap()
    aTloc = nc.dram_tensor("aTloc", [D, NB], FP8, kind="ExternalInput").ap()
    s_out = nc.dram_tensor("s_out", [MT, P, NCH], F32, kind="ExternalOutput").ap()
    diag_out = nc.dram_tensor("diag_out", [1, NB], F32, kind="ExternalOutput").ap()

    with tile.TileContext(nc) as tc:
        with (
            tc.tile_pool(name="persist", bufs=1) as persist,
            tc.tile_pool(name="work", bufs=3) as work,
            tc.tile_pool(name="psp", bufs=6, space="PSUM") as psp,
        ):
            _body(nc, persist, work, psp, qT, aT, aTloc, s_out, diag_out)

    nc.compile()
    return nc


def _body(nc, persist, work, psp, qT, aT, aTloc, s_out, diag_out):
    # Full [128,128] all-ones weight: every lhsT column is 1s, so the
    # ones-matmul writes its column sums broadcast to all 128 output
    # partitions -- the ra chain then runs fat with no DRAM round-trip.
    ones = persist.tile([P, P], BF16, tag="ones")
    nc.vector.memset(ones, 1.0)

    # ---- DMA front: que^T k-tiles interleaved with the first ans chunk so
    # the PE can start within ~2us; later chunks stream behind; the
    # diag-only aTloc slab is deliberately last (off the critical path).
    qts = []
    at_tiles = {}
    for t in range(KT2):
        qt = persist.tile([P, 2, NB], FP8, tag=f"qT{t}")
        nc.sync.dma_start(
            out=qt,
            in_=qT[2 * t * P : (2 * t + 2) * P, :].rearrange("(i p) m -> p i m", i=2),
        )
        qts.append(qt)
        a0 = persist.tile([P, 2, NW], FP8, tag=f"aT{t}_0")
        nc.sync.dma_start(
            out=a0,
            in_=aT[2 * t * P : (2 * t + 2) * P, 0:NW].rearrange(
                "(i p) n -> p i n", i=2
            ),
        )
        at_tiles[(t, 0)] = a0

    # ---- que-norm chain -> per-partition row scale rq = 1/(gamma*qn).
    qn2_ps = psp.tile([P, NW], F32, tag="an2", bufs=2)
    for t in range(KT2):
        sq = work.tile([P, 2, NB], BF16, tag="sq2", bufs=4, name=f"qsq_{t}")
        nc.scalar.square(sq, qts[t])
        sqf = work.tile([P, NB], BF16, tag="sqf", bufs=4, name=f"qsqf_{t}")
        nc.vector.tensor_add(sqf, sq[:, 0, :], sq[:, 1, :])
        nc.tensor.matmul(
            qn2_ps, lhsT=ones, rhs=sqf, start=(t == 0), stop=(t == KT2 - 1)
        )
    # rq = exp(-0.5 * ln(qn2 * gama^2)) = 1/(gama*qn); qn ~ 32 so the
    # reference's max(qn, eps) guard is a no-op for this distribution.
    rq_ln = work.tile([1, NW], F32, tag="ra_ln", bufs=2)
    nc.scalar.activation(rq_ln, qn2_ps[0:1, :], AF.Ln, scale=float(GAMA * GAMA))
    rq_row = persist.tile([1, NW], F32, tag="rq_row")
    nc.scalar.activation(rq_row, rq_ln, AF.Exp, scale=-0.5)
    # Scatter [1,512] -> [128,4] so row scales line up with m-tile partitions.
    rq_sb = persist.tile([P, MT], F32, tag="rq_sb")
    for m in range(MT):
        nc.gpsimd.dma_start(
            out=rq_sb[:, m : m + 1], in_=rq_row[0:1, m * P : (m + 1) * P]
        )

    # ---- Main loop over the 8 column chunks.
    s8 = [persist.tile([P, NCH], F32, tag=f"s8_{m}", name=f"s8_{m}") for m in range(MT)]
    ra_b = []
    for n in range(NCH):
        if n + 1 < NCH:
            for t in range(KT2):
                a = persist.tile(
                    [P, 2, NW], FP8, tag=f"aT{t}_{n + 1}", name=f"aT{t}_{n + 1}"
                )
                nc.sync.dma_start(
                    out=a,
                    in_=aT[
                        2 * t * P : (2 * t + 2) * P, (n + 1) * NW : (n + 2) * NW
                    ].rearrange("(i p) n -> p i n", i=2),
                )
                at_tiles[(t, n + 1)] = a

        # ans-norms for this chunk: an2[j] = sum_d aT[d,j]^2 via square +
        # ones-matmul; then ra = exp(-0.5*ln(an2)) broadcast to 128 rows.
        an2_ps = psp.tile([P, NW], F32, tag="an2", bufs=2, name=f"an2_{n}")
        for t in range(KT2):
            sq = work.tile([P, 2, NW], BF16, tag="sq2", bufs=4, name=f"sq_{n}_{t}")
            if (n * KT2 + t) % 2 == 0:
                nc.scalar.square(sq, at_tiles[(t, n)])
            else:
                nc.vector.tensor_mul(sq, at_tiles[(t, n)], at_tiles[(t, n)])
            sqf = work.tile([P, NW], BF16, tag="sqf", bufs=4, name=f"sqf_{n}_{t}")
            nc.vector.tensor_add(sqf, sq[:, 0, :], sq[:, 1, :])
            nc.tensor.matmul(
                an2_ps, lhsT=ones, rhs=sqf, start=(t == 0), stop=(t == KT2 - 1)
            )
        ra_ln = work.tile([P, NW], F32, tag="ra_ln", bufs=2, name=f"ra_ln_{n}")
        nc.scalar.activation(ra_ln, an2_ps, AF.Ln)
        rb = persist.tile([P, NW], F32, tag=f"ra_b{n}", name=f"ra_b{n}")
        nc.scalar.activation(rb, ra_ln, AF.Exp, scale=-0.5)
        ra_b.append(rb)

        pss = [psp.tile([P, NW], F32, tag="ps", bufs=6, name=f"ps_n{n}_{m}") for m in range(MT)]
        for t in range(KT2):
            for m in range(MT):
                nc.tensor.matmul(
                    pss[m],
                    lhsT=qts[t][:, :, m * P : (m + 1) * P],
                    rhs=at_tiles[(t, n)],
                    start=(t == 0),
                    stop=(t == KT2 - 1),
                    perf_mode=DR,
                )
        for m in range(MT):
            u = work.tile([P, NW], F32, tag="u", name=f"u_{n}_{m}")
            nc.vector.tensor_mul(u, pss[m], ra_b[n])
            nc.scalar.activation(
                u,
                u,
                AF.Exp,
                scale=rq_sb[:, m : m + 1],
                accum_out=s8[m][:, n : n + 1],
            )

    # ---- diagonal: dot(q_i, a_i) via elementwise mul + ones-matmul; scaled
    # by rq_i (gamma folded) and the local 1/an_i. Entirely off-critical.
    atl_tiles = []
    for t in range(KT2):
        atl = work.tile([P, 2, NW], FP8, tag="atl", bufs=2, name=f"atl{t}")
        nc.sync.dma_start(
            out=atl,
            in_=aTloc[2 * t * P : (2 * t + 2) * P, :].rearrange(
                "(i p) n -> p i n", i=2
            ),
        )
        atl_tiles.append(atl)
    al2_ps = psp.tile([P, NW], F32, tag="an2", bufs=2)
    dg_ps = psp.tile([P, NW], F32, tag="an2", bufs=2)
    for t in range(KT2):
        sq = work.tile([P, 2, NW], BF16, tag="sq2", bufs=4, name=f"sqatl_{t}")
        nc.vector.tensor_mul(sq, atl_tiles[t], atl_tiles[t])
        sqf = work.tile([P, NW], BF16, tag="sqf", bufs=4, name=f"sqfatl_{t}")
        nc.vector.tensor_add(sqf, sq[:, 0, :], sq[:, 1, :])
        nc.tensor.matmul(
            al2_ps, lhsT=ones, rhs=sqf, start=(t == 0), stop=(t == KT2 - 1)
        )
        qa = work.tile([P, 2, NW], BF16, tag="qa", bufs=2, name=f"qa_{t}")
        nc.vector.tensor_mul(qa, qts[t], atl_tiles[t])
        qaf = work.tile([P, NW], BF16, tag="qaf", bufs=2, name=f"qaf_{t}")
        nc.vector.tensor_add(qaf, qa[:, 0, :], qa[:, 1, :])
        nc.tensor.matmul(
            dg_ps, lhsT=ones, rhs=qaf, start=(t == 0), stop=(t == KT2 - 1)
        )
    ral_ln = work.tile([1, NW], F32, tag="ra_ln", bufs=2)
    nc.scalar.activation(ral_ln, al2_ps[0:1, :], AF.Ln)
    ral_row = persist.tile([1, NW], F32, tag="ral_row")
    nc.scalar.activation(ral_row, ral_ln, AF.Exp, scale=-0.5)
    diag_row = persist.tile([1, NW], F32, tag="diag_row")
    nc.vector.tensor_mul(diag_row, dg_ps[0:1, :], rq_row)
    nc.vector.tensor_mul(diag_row, diag_row, ral_row)
    nc.sync.dma_start(out=diag_out, in_=diag_row)

    # ---- outputs: raw per-chunk exp-sums [m][128, 8]; host does log+mean.
    for m in range(MT):
        nc.sync.dma_start(out=s_out[m], in_=s8[m])


_CACHE = {}


def _get_program():
    if "nc" not in _CACHE:
        _CACHE["nc"] = _build_program()
    return _CACHE["nc"]


def _make_in_maps(que, ans):
    fp8 = mybir.dt.np(FP8)
    que = np.asarray(que, dtype=np.float32).astype(fp8)
    ans = np.asarray(ans, dtype=np.float32).astype(fp8)
    aT_full = np.ascontiguousarray(ans.T)  # [D, B], shared by all cores
    in_maps = []
    for c in range(NCORES):
        sl = slice(c * NB, (c + 1) * NB)
        in_maps.append(
            {
                "qT": np.ascontiguousarray(que[sl].T),  # [D, NB]
                "aT": aT_full,
                "aTloc": np.ascontiguousarray(ans[sl].T),  # [D, NB]
            }
        )
    return in_maps


def _finish(results):
    # s_out[m, p, n] = sum_j exp(logits) over column chunk n, row m*128+p.
    s = np.concatenate(
        [r["s_out"].sum(axis=-1).reshape(-1) for r in results]
    )  # [B] softmax denominators, local-row order, cores in rank order
    lse = np.log(s)
    diag = np.concatenate([r["diag_out"].reshape(-1) for r in results])
    loss = np.float32(np.mean(lse - diag))
    return np.array([loss], dtype=np.float32)


def kernel(que_batch, ans_batch):
    nc = _get_program()
    in_maps = _make_in_maps(np.asarray(que_batch), np.asarray(ans_batch))
    res = run_bass_kernel_spmd(nc, in_maps, list(range(NCORES)))
    return _finish(res.results)


if __name__ == "__main__":
    rng = np.random.default_rng(0)
    q = rng.standard_normal((B, D), dtype=np.float32)
    a = rng.standard_normal((B, D), dtype=np.float32)
    print(kernel(q, a))



# revision 2
# speedup vs baseline: 2.0440x; 2.0440x over previous
"""Trainium2 Bass kernel for nn_BatchContrastLoss (InfoNCE-style contrastive loss).

Reference computation:
    sim[i,j]  = cos(que_i, ans_j)            (eps-guarded norms)
    logits    = sim / 0.07
    loss      = -mean_i(log_softmax(logits, axis=1)[i,i])

Key restructuring vs the straightforward port: cosine normalization is LINEAR
in each operand, so (q_i/(gama*|q_i|)) . (a_j/|a_j|) == logits_ij exactly.
The row/column norms are folded into the host-side fp8 quantization pass that
already has to touch every element. The device then runs only the two
irreducible parts -- the [B/4, B/2] fp8 GEMM slab and the row-wise
exp-accumulate -- and everything else (log, diagonal dot, mean) stays on the
host where it is O(B*D) noise.

Sharding: 2D (4 que-shards x 2 ans-halves) over 8 cores. Each core reads a
1MB que slab + 2MB ans half (vs 4.5MB for 1D row sharding), computes its
[1024, 2048] logits block, and emits 8 per-row-tile exp-sums. Host pairs the
two ans-halves per row (a trivial add), takes log, subtracts the host-computed
diagonal logits, and means. No cross-core collective (rank-skew stalls cost
more than the 4KB/core of extra host traffic).

Per-core device program (PE-roofline bound, ~27us of DoubleRow fp8 matmul):
  - operands arrive pre-paired d-major [128, 2, *] for DoubleRow e4m3 matmuls
    (2 weights/cell, K=256 per instruction), in exactly the SBUF layout so
    every DMA descriptor is a contiguous 1-2KB per-partition run.
  - m-outer loop with t-major matmuls: each [128, 2, 128] weight tile is
    reused across the 4 column chunks (4x fewer LDWEIGHTS than chunk-outer).
  - PSUM: one 4-bank [128, 4, 512] tile per m, double-buffered; the drain is
    a single ScalarE Exp over all 2048 columns with the fused row-sum
    accumulator -- 8 activation instructions total, hidden under the PE.
"""

import numpy as np

import concourse.bass as bass
import concourse.mybir as mybir
import concourse.tile as tile
from concourse import bacc
from concourse.bass_utils import run_bass_kernel_spmd

# Problem constants (self-contained; the harness provides only the inputs).
B = 4096  # rows of que_batch / ans_batch
D = 1024  # feature dim
NCORES = 8
RSH = 4  # que row shards
CSH = 2  # ans column shards
MB = B // RSH  # local que rows per core = 1024
NB = B // CSH  # local ans cols per core = 2048
P = 128  # SBUF partitions
KT2 = D // (2 * P)  # 4 DoubleRow k-pair tiles (K=256 each)
NW = 512  # column chunk width (one fp32 PSUM bank)
NCH = NB // NW  # 4 column chunks
MT = MB // P  # 8 row tiles of 128
GAMA = 0.07
EPS = 1e-8

F32 = mybir.dt.float32
FP8 = mybir.dt.float8e4  # e4m3: matmul operands; DoubleRow packs 2 weights/cell
DR = mybir.MatmulPerfMode.DoubleRow
AF = mybir.ActivationFunctionType


def _build_program():
    nc = bacc.Bacc(
        "TRN2", target_bir_lowering=False, debug=False, num_devices=NCORES
    )

    # Host-prepped layouts (fp8, DoubleRow-paired, d-major):
    #   qdr[p, t, i, m]     = qhat[m_local, d=(2t+i)*128+p]
    #   adr[p, t, n, i, j2] = ahat[n*512+j2 local, d=(2t+i)*128+p]
    qdr = nc.dram_tensor("qdr", [P, KT2, 2, MB], FP8, kind="ExternalInput").ap()
    adr = nc.dram_tensor("adr", [P, KT2, NCH, 2, NW], FP8, kind="ExternalInput").ap()
    s_out = nc.dram_tensor("s_out", [P, MT], F32, kind="ExternalOutput").ap()

    with tile.TileContext(nc) as tc:
        with (
            tc.tile_pool(name="persist", bufs=1) as persist,
            tc.tile_pool(name="work", bufs=2) as work,
            tc.tile_pool(name="psp", bufs=2, space="PSUM") as psp,
        ):
            _body(nc, persist, work, psp, qdr, adr, s_out)

    nc.compile()
    return nc


def _body(nc, persist, work, psp, qdr, adr, s_out):
    # ---- DMA front: every transfer is contiguous per partition. Issue in
    # the order the m=0 iteration consumes it so the PE unblocks after the
    # first ~640KB.
    qts = []
    ats = {}
    for t in range(KT2):
        qt = persist.tile([P, 2, MB], FP8, tag=f"q{t}", name=f"q{t}")
        nc.sync.dma_start(out=qt, in_=qdr[:, t])
        qts.append(qt)
        for n in range(NCH):
            a = persist.tile([P, 2, NW], FP8, tag=f"a{t}_{n}", name=f"a{t}_{n}")
            nc.sync.dma_start(out=a, in_=adr[:, t, n])
            ats[(t, n)] = a

    s_sb = persist.tile([P, MT], F32, tag="s_sb")

    # ---- main loop: 8 row tiles, 16 weight loads of 4 matmuls each.
    for m in range(MT):
        ps = psp.tile([P, NCH, NW], F32, tag="ps", bufs=2, name=f"ps_{m}")
        for t in range(KT2):
            w = qts[t][:, :, m * P : (m + 1) * P]
            for n in range(NCH):
                nc.tensor.matmul(
                    ps[:, n],
                    lhsT=w,
                    rhs=ats[(t, n)],
                    start=(t == 0),
                    stop=(t == KT2 - 1),
                    perf_mode=DR,
                )
        # drain: one Exp over all 4 banks with fused row-sum accumulation.
        scr = work.tile([P, NCH, NW], F32, tag="scr", bufs=2, name=f"scr_{m}")
        nc.scalar.activation(
            scr, ps, AF.Exp, accum_out=s_sb[:, m : m + 1]
        )

    nc.sync.dma_start(out=s_out, in_=s_sb)


_CACHE = {}


def _get_program():
    if "nc" not in _CACHE:
        _CACHE["nc"] = _build_program()
    return _CACHE["nc"]


def _prep(que, ans):
    """Normalize (norm folding), quantize to fp8, lay out for DoubleRow DMA.

    Returns (in_maps, diag) where diag[i] = qhat_i . ahat_i computed from the
    exact fp8 values the device multiplies (f32 accumulation, same as PSUM).
    """
    fp8 = mybir.dt.np(FP8)
    que = np.asarray(que, dtype=np.float32)
    ans = np.asarray(ans, dtype=np.float32)
    qn = np.maximum(np.sqrt(np.einsum("id,id->i", que, que)), EPS)
    an = np.maximum(np.sqrt(np.einsum("id,id->i", ans, ans)), EPS)
    qhat = (que / (np.float32(GAMA) * qn)[:, None]).astype(fp8)
    ahat = (ans / an[:, None]).astype(fp8)

    qf = qhat.astype(np.float32)
    af = ahat.astype(np.float32)
    diag = np.einsum("id,id->i", qf, af)  # logits diagonal, bit-compatible

    in_maps = []
    for cid in range(NCORES):
        r, c = divmod(cid, CSH)
        qslab = qhat[r * MB : (r + 1) * MB]  # [MB, D]
        aslab = ahat[c * NB : (c + 1) * NB]  # [NB, D]
        # [D, MB] -> [KT2, 2, P, MB] -> [P, KT2, 2, MB]
        qdr = np.ascontiguousarray(
            qslab.T.reshape(KT2, 2, P, MB).transpose(2, 0, 1, 3)
        )
        # [D, NB] -> [KT2, 2, P, NCH, NW] -> [P, KT2, NCH, 2, NW]
        adr = np.ascontiguousarray(
            aslab.T.reshape(KT2, 2, P, NCH, NW).transpose(2, 0, 3, 1, 4)
        )
        in_maps.append({"qdr": qdr, "adr": adr})
    return in_maps, diag


def _finish(results, diag):
    # s_out[p, m] = sum_j exp(logits) over this core's ans half, row m*128+p.
    s = np.zeros(B, dtype=np.float64)
    for cid, res in enumerate(results):
        r, _ = divmod(cid, CSH)
        so = np.asarray(res["s_out"], dtype=np.float64)  # [P, MT]
        rows = r * MB + np.arange(MT) * P  # row base per m
        for m in range(MT):
            s[rows[m] : rows[m] + P] += so[:, m]
    loss = np.float32(np.mean(np.log(s) - diag))
    return np.array([loss], dtype=np.float32)


def kernel(que_batch, ans_batch):
    nc = _get_program()
    in_maps, diag = _prep(que_batch, ans_batch)
    res = run_bass_kernel_spmd(nc, in_maps, list(range(NCORES)))
    return _finish(res.results, diag)


if __name__ == "__main__":
    rng = np.random.default_rng(0)
    q = rng.standard_normal((B, D), dtype=np.float32)
    a = rng.standard_normal((B, D), dtype=np.float32)
    print(kernel(q, a))
